# revision 17
# baseline (speedup 1.0000x reference)
"""Trainium2 Bass kernel for the DEN-layer Mahalanobis problem.

Computes mah[b, e] = (x_b - c_e)^T Sigma_e^{-1} (x_b - c_e) for
B=8192, E=32, D=256, returning [B, E] float32.

Strategy
--------
The Sigmas are I + (A A^T)/D with A ~ 0.1*randn, so A_e = Sigma_e^{-1}
has eigenvalues confined to a narrow band (measured [0.94, 1.0]).  Host
eigendecomposition splits each A_e into a scalar multiple of I plus a
low-rank correction:

  A_e = alpha_e I - G_e G_e^T + F_e,   G_e = V_kept sqrt(alpha_e - lam_kept)

where the dropped eigenvalue band is folded into alpha_e (band midpoint)
and the residual F_e has spectral norm <= delta_e (band half-width).
This gives a CERTIFIED pointwise bound valid for every input x:

  |mah_approx - mah| <= delta_e ||dif||^2   and  mah >= lam_min ||dif||^2
  =>  rel err <= delta_e / lam_min   (asserted < CERT_MAX at prep time)

With rank R=32 per e the certificate is ~1.2e-2 and the empirical error
on the actual inputs is ~6.6e-3 (gate: 2e-2).

Everything except ||G_e^T x||^2 is affine in per-sample host-cheap terms
and folds into a per-(b,e) correction computed on host (same boundary as
the previous kernel, which hosted kconst - 2 x.u):

  mah[b,e] = corr[b,e] - sum_j (x_b @ G_e)_j^2
  corr[b,e] = alpha_e ||x_b||^2 + x_b . w_e + const_e

Device (data parallel over B, 8 cores, B_loc=1024, blocks of 128 rows):
  - PE: Y = x @ G for all e, G packed 512 cols/bank (GPB e's per bank),
    contraction over d in 2 halves sharing the x^T stationary block.
  - Scalar: Square per bank, PSUM -> SBUF ((172+512)/1.2 = 570 ns) — the
    grouped-bn_stats route is rejected by this walrus (BNStats output must
    be exactly 6/partition), and per-e bn_stats/accum pay a ~300-600 ns
    fixed cost per value.  A dummy Square at t=0 pulls the one-time ACT
    table load into the DMA head.
  - Vector: grouped tensor_reduce [128, GPB, R] -> [128, GPB] per bank
    ((58+512)/0.96 = 594 ns), giving s1 = sum_j Y^2 per e directly.
  - GpSimd: res = corr - s1, plus DMA issue.
"""

import numpy as np
import ml_dtypes

import concourse.bass as bass
import concourse.mybir as mybir
import concourse.tile as tile
from concourse.bass_utils import run_bass_kernel_spmd

E, B, D = 32, 8192, 256
N_CORES = 8
B_LOC = B // N_CORES          # 1024 rows per core
NBB = B_LOC // 128            # 8 row blocks per core
P = 128

R = 32                        # rank kept per e (32*E/512 banks per block)
NBANK = (E * R) // 512        # PSUM banks per block
GPB = E // NBANK              # e's (bn_stats groups) per bank
NPAR = R // 2                 # elements per parity within a group
CERT_MAX = 0.016              # certified rel-err bound must stay under this

F32 = mybir.dt.float32
BF16 = mybir.dt.bfloat16


def _split_multi_waits(nc, limit=1):
    """This walrus build accepts only one sync wait per instruction
    (setupSyncWait raises "Too many sync wait commands" for >=2). Tile
    freely attaches several. Spill all but the last wait onto preceding
    single-wait NoOps on the same engine; engine program order makes this
    equivalent."""
    for fn in nc.m.functions:
        for bb in fn.blocks:
            new_list = []
            changed = False
            for inst in bb.instructions:
                si = inst.sync_info
                if si is not None and len(si.on_wait) > limit:
                    waits = list(si.on_wait)
                    for j, w in enumerate(waits[:-limit]):
                        new_list.append(
                            mybir.InstNoOp(
                                name=f"{inst.name}-ws{j}",
                                engine=inst.engine,
                                sync_info=mybir.SyncInfo(on_wait=[w], on_update=[]),
                                text_hint="waitsplit",
                                bass_nofuse=True,
                            )
                        )
                    inst.sync_info = mybir.SyncInfo(
                        on_wait=waits[-limit:], on_update=list(si.on_update)
                    )
                    changed = True
                new_list.append(inst)
            if changed:
                bb.instructions[:] = new_list


def _build_program():
    nc = bass.Bass("TRN2", target_bir_lowering=False, debug=False,
                   num_devices=N_CORES)

    xt_d = nc.dram_tensor("xt_in", [P, 2, B_LOC], BF16, kind="ExternalInput")
    g_d = nc.dram_tensor("g_in", [P, 2 * NBANK, 512], BF16, kind="ExternalInput")
    corr_d = nc.dram_tensor("corr_in", [P, NBB, E], F32, kind="ExternalInput")
    out_d = nc.dram_tensor("mah_out", [P, NBB * E], F32, kind="ExternalOutput")

    mul = mybir.AluOpType.mult
    sub = mybir.AluOpType.subtract
    add = mybir.AluOpType.add

    with tile.TileContext(nc) as tc:
        with (
            tc.tile_pool(name="const", bufs=1) as const,
            tc.tile_pool(name="ypsum", bufs=2 * NBANK, space="PSUM") as ypsum,
            tc.tile_pool(name="warmpsum", bufs=1, space="PSUM") as warmpsum,
            tc.tile_pool(name="sqp", bufs=2 * NBANK) as sqp,
            tc.tile_pool(name="resp", bufs=4) as resp,
        ):
            wact = const.tile([P, 1], F32, tag="wact")
            wsrc = const.tile([P, 512], BF16, tag="wsrc")
            nc.gpsimd.memset(wact[:], 0.0)
            nc.gpsimd.memset(wsrc[:], 0.0078125)

            xt = const.tile([P, 2, B_LOC], BF16, tag="xt")
            g_sb = const.tile([P, 2, NBANK, 512], BF16, tag="g")
            corr_sb = const.tile([P, NBB, E], F32, tag="corr")
            resall = const.tile([P, NBB, E], F32, tag="resall")

            # Input DMAs spread over four engine queues, ordered by when
            # block 0 (then later blocks) needs each chunk.
            nc.sync.dma_start(xt[:, :, 0:256], xt_d[:, :, 0:256])
            for q in range(NBANK):
                nc.scalar.dma_start(g_sb[:, 0, q, :], g_d[:, 0 * NBANK + q, :])
                nc.sync.dma_start(g_sb[:, 1, q, :], g_d[:, 1 * NBANK + q, :])
            nc.sync.dma_start(xt[:, :, 256:512], xt_d[:, :, 256:512])
            nc.gpsimd.dma_start(corr_sb[:], corr_d[:])
            nc.gpsimd.dma_start(xt[:, :, 512:1024], xt_d[:, :, 512:1024])

            # dummy Square: pulls the one-time ACT table load into the DMA
            # head instead of stalling block 0's first real activation.
            nc.scalar.activation(wact[:], wact[:],
                                 mybir.ActivationFunctionType.Square)

            # PE warmup: matmuls on memset garbage — no DMA dependency, so
            # the PE is busy from the moment it leaves the preamble (HAM
            # needs ~3.4us of sustained activity to unthrottle 1.2->2.4GHz).
            # Small N=128 tiles so the first real matmul slots in quickly.
            yw = warmpsum.tile([P, 512], F32, tag="yw")
            for _ in range(16):
                nc.tensor.matmul(yw[:, 0:P], lhsT=wsrc[:, 0:P],
                                 rhs=wsrc[:, 0:P], start=True, stop=True)

            for bb in range(NBB):
                bbs = bass.ts(bb, P)
                ys = [ypsum.tile([P, GPB, R], F32, name=f"y{q}", tag="y")
                      for q in range(NBANK)]
                for h in range(2):
                    for q in range(NBANK):
                        nc.tensor.matmul(ys[q][:, :, :], lhsT=xt[:, h, bbs],
                                         rhs=g_sb[:, h, q, :],
                                         start=(h == 0), stop=(h == 1))

                s1 = resp.tile([P, E], F32, tag="s1")
                for q in range(NBANK):
                    sq = sqp.tile([P, GPB, R], F32, name=f"sq{q}", tag="sq")
                    nc.scalar.activation(
                        sq[:, :, :], ys[q][:, :, :],
                        mybir.ActivationFunctionType.Square)
                    nc.vector.tensor_reduce(
                        s1[:, q * GPB:(q + 1) * GPB], sq[:, :, :],
                        mybir.AxisListType.X, add)

                nc.gpsimd.tensor_tensor(resall[:, bb, :], corr_sb[:, bb, :],
                                        s1[:, :], sub)

            nc.sync.dma_start(out_d[:, :], resall[:, :, :])

    _split_multi_waits(nc)
    return nc


_PROGRAM = None


def _host_prep(x, Centroids, Sigmas):
    """Returns per-core input maps."""
    x64 = np.asarray(x, dtype=np.float64)
    c = np.asarray(Centroids, dtype=np.float64).reshape(E, D)
    sig = np.asarray(Sigmas, dtype=np.float64)
    inv = np.linalg.inv(sig)
    inv = 0.5 * (inv + inv.transpose(0, 2, 1))
    lam, V = np.linalg.eigh(inv)                   # [E, D] asc, [E, D, D]

    alpha = 0.5 * (lam[:, R] + lam[:, -1])         # dropped-band midpoint
    delta = 0.5 * (lam[:, -1] - lam[:, R])
    cert = float((delta / lam[:, 0]).max())
    assert cert < CERT_MAX, (
        f"certified rel-err bound {cert:.4f} exceeds {CERT_MAX}; "
        "rank R too small for these Sigmas")

    G = V[:, :, :R] * np.sqrt(alpha[:, None, None] - lam[:, None, :R])
    Gb = G.astype(ml_dtypes.bfloat16).astype(np.float64)   # device-rounded G

    g = np.zeros((P, 2 * NBANK, 512), dtype=np.float64)
    for e in range(E):
        q, i = e // GPB, e % GPB
        g[:, q, i * R:(i + 1) * R] = Gb[e, :P, :]
        g[:, NBANK + q, i * R:(i + 1) * R] = Gb[e, P:, :]
    g = np.ascontiguousarray(g).astype(ml_dtypes.bfloat16)

    # affine part, exact in fp64 (uses the device-rounded G for consistency)
    S2 = (x64 * x64).sum(1)                        # [B]
    GtC = np.einsum('edr,ed->er', Gb, c)           # [E, R]
    w = -2.0 * alpha[:, None] * c + 2.0 * np.einsum('edr,er->ed', Gb, GtC)
    const = alpha * (c * c).sum(1) - (GtC * GtC).sum(1)
    corr_full = alpha[None, :] * S2[:, None] + x64 @ w.T + const[None, :]

    xb = x64.astype(ml_dtypes.bfloat16)
    in_maps = []
    for i in range(N_CORES):
        sl = slice(i * B_LOC, (i + 1) * B_LOC)
        # [P, 2, B_LOC]: partition p holds d-rows p (half 0) and 128+p (half 1)
        xt = np.ascontiguousarray(
            np.ascontiguousarray(xb[sl].T).reshape(2, P, B_LOC).transpose(1, 0, 2))
        corr = corr_full[sl].astype(np.float32)
        corr = np.ascontiguousarray(corr.reshape(NBB, P, E).transpose(1, 0, 2))
        in_maps.append({"xt_in": xt, "g_in": g, "corr_in": corr})
    return in_maps


def kernel(x, Centroids, Sigmas):
    global _PROGRAM
    if _PROGRAM is None:
        _PROGRAM = _build_program()
    in_maps = _host_prep(x, Centroids, Sigmas)
    res = run_bass_kernel_spmd(_PROGRAM, in_maps, list(range(N_CORES)))
    out = np.concatenate(
        [res.results[i]["mah_out"].reshape(P, NBB, E).transpose(1, 0, 2)
         .reshape(B_LOC, E) for i in range(N_CORES)],
        axis=0,
    )
    return np.ascontiguousarray(out.astype(np.float32))
